# revision 8
# baseline (speedup 1.0000x reference)
"""Differential self-attention head on 8 Trainium2 NeuronCores.

Sharding: 8 cores = 4 batches x 2 softmax branches. Core c handles batch
c//2 and branch c%2 (branch 0 -> (Wq1, Wk1), branch 1 -> (Wq2, Wk2)).
Every core runs the identical SPMD program over its own data:

  - projections q,k,v with bias folded in via an augmented contraction
    (E=1024 data rows + 1 ones-row + pad to 1152 = 9 chunks of 128)
  - causal scores computed transposed [k, q] so exp(S) is directly the
    moving operand of the v^T @ p matmul (no on-chip transpose of p)
  - exp on ScalarE straight from PSUM with scale=1/sqrt(D)
  - diagonal-tile causal masking via a multiply with host-built 0/1 tiles
  - row sums via ones-vector matmuls accumulated in PSUM
  - outputs the unnormalized numerator num = v^T @ p [D, S] and the
    denominator d [1, S]; the host divides and combines the two branches
    (o = num1/d1 - lam*num2/d2) and transposes back to [S, D].

All matmul operands are fp16 (measured end-to-end rel err ~7e-4);
accumulation is fp32 in PSUM.
"""

import sys

import numpy as np

for _p in ("/opt/trn_rl_repo",):
    if _p not in sys.path:
        sys.path.insert(0, _p)

B, S, E, D = 4, 4096, 1024, 128
EA = 1152  # augmented contraction: E + ones row, padded to 9*128
QB = 512  # query block (matmul moving free dim)
KT = 128  # key tile (partition dim of transposed scores)

_PROG_CACHE = {}
LAST_RUN = None  # BassKernelResults of the most recent kernel() call


def _build_program(s, ea, qb, kt):
    import concourse.bass as bass  # noqa: F401
    import concourse.mybir as mybir
    from concourse import bacc
    from concourse.tile import TileContext
    from concourse.masks import make_identity

    fp16 = mybir.dt.float16
    fp32 = mybir.dt.float32
    n_ec = ea // 128  # contraction chunks
    n_sb = s // qb  # 512-wide column blocks of the full sequence
    n_qb = s // qb  # query blocks
    n_st = s // kt  # 128-row key/seq tiles
    npair = qb // kt  # diag mask variants (kt tiles per query block)

    nc = bacc.Bacc("TRN2", target_bir_lowering=False, debug=False)
    xT = nc.dram_tensor("xT", [ea, s], fp16, kind="ExternalInput")
    wq = nc.dram_tensor("wq", [ea, D], fp16, kind="ExternalInput")
    wk = nc.dram_tensor("wk", [ea, D], fp16, kind="ExternalInput")
    wv = nc.dram_tensor("wv", [ea, D], fp16, kind="ExternalInput")
    dmask = nc.dram_tensor("dmask", [128, npair * qb], fp16, kind="ExternalInput")
    num_out = nc.dram_tensor("num", [D, s], fp32, kind="ExternalOutput")
    den_out = nc.dram_tensor("den", [1, s], fp32, kind="ExternalOutput")

    inv = 1.0 / np.sqrt(np.float32(D))

    with TileContext(nc) as tc:
        with (
            tc.tile_pool(name="const", bufs=1) as const_pool,
            tc.tile_pool(name="acts", bufs=1) as acts_pool,
            tc.tile_pool(name="ptiles", bufs=18) as p_pool,
            tc.tile_pool(name="outs", bufs=3) as out_pool,
        ):
            # ---- constants ----
            w_sb = const_pool.tile([128, n_ec, 3 * D], fp16, name="w_sb")
            nc.sync.dma_start(
                out=w_sb[:, :, 0:D], in_=wq.rearrange("(c p) d -> p c d", p=128)
            )
            nc.sync.dma_start(
                out=w_sb[:, :, D : 2 * D], in_=wk.rearrange("(c p) d -> p c d", p=128)
            )
            nc.sync.dma_start(
                out=w_sb[:, :, 2 * D : 3 * D],
                in_=wv.rearrange("(c p) d -> p c d", p=128),
            )
            mask_sb = const_pool.tile([128, npair * qb], fp16, name="mask_sb")
            nc.sync.dma_start(out=mask_sb, in_=dmask[:, :])
            ones_sb = const_pool.tile([128, 1], fp16, name="ones_sb")
            nc.vector.memset(ones_sb, 1.0)
            ident = const_pool.tile([128, 128], fp16, name="ident")
            make_identity(nc, ident)

            # ---- x^T staging ----
            xt_sb = acts_pool.tile([128, n_ec, s], fp16, name="xt_sb")
            for c in range(n_ec):
                nc.sync.dma_start(
                    out=xt_sb[:, c, :], in_=xT[c * 128 : (c + 1) * 128, :]
                )

            # ---- projections (qT, kT, vT in [D, s] layout) ----
            qT = acts_pool.tile([128, s], fp16, name="qT")
            kTt = acts_pool.tile([128, s], fp16, name="kTt")
            v_sb = acts_pool.tile([128, n_st, D], fp16, name="v_sb")
            with (
                tc.tile_pool(name="proj_ps", bufs=2, space="PSUM") as proj_ps,
                tc.tile_pool(name="tr_ps", bufs=2, space="PSUM") as tr_ps,
            ):
                vT = acts_pool.tile([128, s], fp16, name="vT")
                for mi, dst in ((0, qT), (1, kTt), (2, vT)):
                    for sb in range(n_sb):
                        ps = proj_ps.tile([128, qb], fp32, name="ps", tag="ps")
                        for c in range(n_ec):
                            nc.tensor.matmul(
                                ps,
                                lhsT=w_sb[:, c, mi * D : (mi + 1) * D],
                                rhs=xt_sb[:, c, sb * qb : (sb + 1) * qb],
                                start=(c == 0),
                                stop=(c == n_ec - 1),
                            )
                        nc.vector.tensor_copy(dst[:, sb * qb : (sb + 1) * qb], ps)
                # v natural layout [s, D] via PE transposes of vT
                for st in range(n_st):
                    tp = tr_ps.tile([128, 128], fp16, name="tp", tag="tp")
                    nc.tensor.transpose(
                        tp, vT[:, st * 128 : (st + 1) * 128], ident
                    )
                    nc.vector.tensor_copy(v_sb[:, st, :], tp)

            # ---- attention ----
            den_sb = out_pool.tile([1, s], fp32, name="den_sb", bufs=1)
            with (
                tc.tile_pool(name="s_ps", bufs=2, space="PSUM") as s_ps,
                tc.tile_pool(name="num_ps", bufs=2, space="PSUM") as num_ps,
                tc.tile_pool(name="d_ps", bufs=2, space="PSUM") as d_ps,
            ):
                for qbi in range(n_qb):
                    nkt = (qbi + 1) * (qb // kt)  # causal: key tiles needed
                    qs = slice(qbi * qb, (qbi + 1) * qb)
                    nump = num_ps.tile([128, qb], fp32, name="nump", tag="nump")
                    dp = d_ps.tile([1, qb], fp32, name="dp", tag="dp")
                    pts = []
                    for ktp in range(nkt // 2):
                        k0 = 2 * ktp
                        sp = s_ps.tile([128, 2 * qb], fp32, name="sp", tag="sp")
                        pt = p_pool.tile([128, 2 * qb], fp16, name="pt", tag="pt")
                        pts.append(pt)
                        for h in range(2):
                            ktile = k0 + h
                            nc.tensor.matmul(
                                sp[:, h * qb : (h + 1) * qb],
                                lhsT=kTt[:, ktile * kt : (ktile + 1) * kt],
                                rhs=qT[:, qs],
                                start=True,
                                stop=True,
                            )
                        nc.scalar.activation(
                            pt, sp, mybir.ActivationFunctionType.Exp, scale=float(inv)
                        )
                        if k0 + 2 == nkt or k0 + 4 == nkt:
                            # last two tile-pairs of this query block sit on
                            # the causal diagonal: zero out k > q entries
                            j0 = k0 - (nkt - npair)
                            nc.vector.tensor_mul(
                                pt, pt, mask_sb[:, j0 * qb : (j0 + 2) * qb]
                            )
                        for h in range(2):
                            ktile = k0 + h
                            nc.tensor.matmul(
                                nump,
                                lhsT=v_sb[:, ktile, :],
                                rhs=pt[:, h * qb : (h + 1) * qb],
                                start=(ktile == 0),
                                stop=(ktile == nkt - 1),
                            )
                    # row sums: ones-vector matmuls, stationary reused
                    for ktile in range(nkt):
                        pt = pts[ktile // 2]
                        h = ktile % 2
                        nc.tensor.matmul(
                            dp,
                            lhsT=ones_sb,
                            rhs=pt[:, h * qb : (h + 1) * qb],
                            start=(ktile == 0),
                            stop=(ktile == nkt - 1),
                        )
                    numo = out_pool.tile([128, qb], fp32, name="numo", tag="numo")
                    nc.vector.tensor_copy(numo, nump)
                    nc.sync.dma_start(out=num_out[:, qs], in_=numo)
                    nc.vector.tensor_copy(den_sb[:, qs], dp)
                nc.sync.dma_start(out=den_out[:, :], in_=den_sb)
    nc.compile()
    return nc


def _prep_inputs(x, Wq1, bq1, Wq2, bq2, Wk1, bk1, Wk2, bk2, Wv, bv):
    """Host-side data prep: augmented fp16 transposed activations + weights."""
    x = np.asarray(x, dtype=np.float32)
    xT = np.zeros((B, EA, S), dtype=np.float16)
    xT[:, :E, :] = x.transpose(0, 2, 1).astype(np.float16)
    xT[:, E, :] = 1.0  # ones row: folds the bias into the matmul

    def aug(W, b):
        Wa = np.zeros((EA, D), dtype=np.float16)
        Wa[:E] = np.asarray(W, dtype=np.float32).astype(np.float16)
        Wa[E] = np.asarray(b, dtype=np.float32).astype(np.float16)
        return Wa

    wq_br = [aug(Wq1, bq1), aug(Wq2, bq2)]
    wk_br = [aug(Wk1, bk1), aug(Wk2, bk2)]
    wv_a = aug(Wv, bv)

    # 0/1 masks for the diagonal tile-pairs, [128, 4*512] fp16:
    # variant j (kt = qb*4 + j): valid iff q_local >= j*128 + k_local
    ki = np.arange(KT)[:, None]
    qi = np.arange(QB)[None, :]
    dm = np.zeros((128, (QB // KT) * QB), dtype=np.float16)
    for j in range(QB // KT):
        dm[:, j * QB : (j + 1) * QB] = (qi >= j * KT + ki).astype(np.float16)
    return xT, wq_br, wk_br, wv_a, dm


def kernel(x, Wq1, bq1, Wq2, bq2, Wk1, bk1, Wk2, bk2, Wv, bv, lam, mask):
    from concourse.bass_utils import run_bass_kernel_spmd

    xT, wq_br, wk_br, wv_a, dm = _prep_inputs(
        x, Wq1, bq1, Wq2, bq2, Wk1, bk1, Wk2, bk2, Wv, bv
    )

    key = (S, EA, QB, KT)
    if key not in _PROG_CACHE:
        _PROG_CACHE[key] = _build_program(*key)
    nc = _PROG_CACHE[key]

    in_maps = []
    for c in range(8):
        b, br = c // 2, c % 2
        in_maps.append(
            {
                "xT": np.ascontiguousarray(xT[b]),
                "wq": wq_br[br],
                "wk": wk_br[br],
                "wv": wv_a,
                "dmask": dm,
            }
        )
    run = run_bass_kernel_spmd(nc, in_maps, core_ids=list(range(8)))
    global LAST_RUN
    LAST_RUN = run
    res = run.results

    lam = np.float32(np.asarray(lam))
    out = np.empty((B, S, D), dtype=np.float32)
    for b in range(B):
        n1, d1 = res[2 * b]["num"], res[2 * b]["den"]
        n2, d2 = res[2 * b + 1]["num"], res[2 * b + 1]["den"]
        out[b] = (n1 / d1 - lam * (n2 / d2)).T
    return out


# revision 12
# speedup vs baseline: 1.1683x; 1.1683x over previous
"""Differential self-attention head on 8 Trainium2 NeuronCores.

Sharding: 8 cores = 4 batches x 2 softmax branches. Core c handles batch
c//2 and branch c%2 (branch 0 -> (Wq1, Wk1), branch 1 -> (Wq2, Wk2)).
Every core runs the identical SPMD program over its own data:

  - projections q,k,v with bias folded in via an augmented contraction
    (E=1024 data rows + 1 ones-row + pad to 1152 = 9 chunks of 128)
  - causal scores computed transposed [k, q] so exp(S) is directly the
    moving operand of the v^T @ p matmul (no on-chip transpose of p)
  - exp on ScalarE straight from PSUM with scale=1/sqrt(D)
  - diagonal-tile causal masking via a multiply with host-built 0/1 tiles
  - row sums via ones-vector matmuls accumulated in PSUM
  - outputs the unnormalized numerator num = v^T @ p [D, S] and the
    denominator d [1, S]; the host divides and combines the two branches
    (o = num1/d1 - lam*num2/d2) and transposes back to [S, D].

All matmul operands are fp16 (measured end-to-end rel err ~7e-4);
accumulation is fp32 in PSUM.
"""

import sys

import numpy as np

for _p in ("/opt/trn_rl_repo",):
    if _p not in sys.path:
        sys.path.insert(0, _p)

B, S, E, D = 4, 4096, 1024, 128
EA = 1152  # augmented contraction: E + ones row, padded to 9*128
QB = 512  # query block (matmul moving free dim)
KT = 128  # key tile (partition dim of transposed scores)

_PROG_CACHE = {}
LAST_RUN = None  # BassKernelResults of the most recent kernel() call


def _build_program(s, ea, qb, kt):
    import concourse.bass as bass  # noqa: F401
    import concourse.mybir as mybir
    from concourse import bacc
    from concourse.tile import TileContext
    from concourse.masks import make_identity

    fp16 = mybir.dt.float16
    fp32 = mybir.dt.float32
    n_ec = ea // 128  # contraction chunks
    n_sb = s // qb  # 512-wide column blocks of the full sequence
    n_qb = s // qb  # query blocks
    n_st = s // kt  # 128-row key/seq tiles
    npair = qb // kt  # diag mask variants (kt tiles per query block)

    nc = bacc.Bacc("TRN2", target_bir_lowering=False, debug=False)
    xT = nc.dram_tensor("xT", [ea, s], fp16, kind="ExternalInput")
    wq = nc.dram_tensor("wq", [ea, D], fp16, kind="ExternalInput")
    wk = nc.dram_tensor("wk", [ea, D], fp16, kind="ExternalInput")
    wv = nc.dram_tensor("wv", [ea, D], fp16, kind="ExternalInput")
    dmask = nc.dram_tensor("dmask", [128, npair * qb], fp16, kind="ExternalInput")
    num_out = nc.dram_tensor("num", [D, s], fp32, kind="ExternalOutput")
    den_out = nc.dram_tensor("den", [1, s], fp32, kind="ExternalOutput")

    inv = 1.0 / np.sqrt(np.float32(D))

    with TileContext(nc) as tc:
        with (
            tc.tile_pool(name="const", bufs=1) as const_pool,
            tc.tile_pool(name="acts", bufs=1) as acts_pool,
            tc.tile_pool(name="ptiles", bufs=18) as p_pool,
            tc.tile_pool(name="outs", bufs=3) as out_pool,
        ):
            # ---- constants ----
            w_sb = const_pool.tile([128, n_ec, 3 * D], fp16, name="w_sb")
            nc.sync.dma_start(
                out=w_sb[:, :, 0:D], in_=wq.rearrange("(c p) d -> p c d", p=128)
            )
            nc.sync.dma_start(
                out=w_sb[:, :, D : 2 * D], in_=wk.rearrange("(c p) d -> p c d", p=128)
            )
            nc.sync.dma_start(
                out=w_sb[:, :, 2 * D : 3 * D],
                in_=wv.rearrange("(c p) d -> p c d", p=128),
            )
            mask_sb = const_pool.tile([128, npair * qb], fp16, name="mask_sb")
            nc.sync.dma_start(out=mask_sb, in_=dmask[:, :])
            ones_sb = const_pool.tile([128, 1], fp16, name="ones_sb")
            nc.vector.memset(ones_sb, 1.0)
            ident = const_pool.tile([128, 128], fp16, name="ident")
            make_identity(nc, ident)

            # ---- x^T staging: column-blocked so projections (and then
            # attention) can start after the first 512-column slice lands
            # instead of after the full 8-9 MB transfer ----
            xt_sb = acts_pool.tile([128, n_ec, s], fp16, name="xt_sb")
            xT_r = xT.rearrange("(c p) s -> p c s", p=128)
            for sb in range(n_sb):
                nc.sync.dma_start(
                    out=xt_sb[:, :, sb * qb : (sb + 1) * qb],
                    in_=xT_r[:, :, sb * qb : (sb + 1) * qb],
                )

            # ---- projections (qT, kT, vT in [D, s] layout), sb-outer so
            # each column block completes as soon as its DMA lands ----
            qT = acts_pool.tile([128, s], fp16, name="qT")
            kTt = acts_pool.tile([128, s], fp16, name="kTt")
            v_sb = acts_pool.tile([128, n_st, D], fp16, name="v_sb")
            with (
                tc.tile_pool(name="proj_ps", bufs=2, space="PSUM") as proj_ps,
                tc.tile_pool(name="tr_ps", bufs=2, space="PSUM") as tr_ps,
            ):
                vT = acts_pool.tile([128, s], fp16, name="vT")
                for sb in range(n_sb):
                    for mi, dst in ((0, qT), (1, kTt), (2, vT)):
                        ps = proj_ps.tile([128, qb], fp32, name="ps", tag="ps")
                        for c in range(n_ec):
                            nc.tensor.matmul(
                                ps,
                                lhsT=w_sb[:, c, mi * D : (mi + 1) * D],
                                rhs=xt_sb[:, c, sb * qb : (sb + 1) * qb],
                                start=(c == 0),
                                stop=(c == n_ec - 1),
                            )
                        nc.vector.tensor_copy(dst[:, sb * qb : (sb + 1) * qb], ps)
                    # v natural layout [s, D] via PE transposes of vT
                    for j in range(qb // 128):
                        st = sb * (qb // 128) + j
                        tp = tr_ps.tile([128, 128], fp16, name="tp", tag="tp")
                        nc.tensor.transpose(
                            tp, vT[:, st * 128 : (st + 1) * 128], ident
                        )
                        nc.vector.tensor_copy(v_sb[:, st, :], tp)

            # ---- attention ----
            den_sb = out_pool.tile([1, s], fp32, name="den_sb", bufs=1)
            with (
                tc.tile_pool(name="s_ps", bufs=2, space="PSUM") as s_ps,
                tc.tile_pool(name="num_ps", bufs=2, space="PSUM") as num_ps,
                tc.tile_pool(name="d_ps", bufs=2, space="PSUM") as d_ps,
            ):
                for qbi in range(n_qb):
                    nkt = (qbi + 1) * (qb // kt)  # causal: key tiles needed
                    qs = slice(qbi * qb, (qbi + 1) * qb)
                    nump = num_ps.tile([128, qb], fp32, name="nump", tag="nump")
                    dp = d_ps.tile([1, qb], fp32, name="dp", tag="dp")
                    pts = []
                    for ktp in range(nkt // 2):
                        k0 = 2 * ktp
                        sp = s_ps.tile([128, 2 * qb], fp32, name="sp", tag="sp")
                        pt = p_pool.tile([128, 2 * qb], fp16, name="pt", tag="pt")
                        pts.append(pt)
                        for h in range(2):
                            ktile = k0 + h
                            nc.tensor.matmul(
                                sp[:, h * qb : (h + 1) * qb],
                                lhsT=kTt[:, ktile * kt : (ktile + 1) * kt],
                                rhs=qT[:, qs],
                                start=True,
                                stop=True,
                            )
                        nc.scalar.activation(
                            pt, sp, mybir.ActivationFunctionType.Exp, scale=float(inv)
                        )
                        if k0 + 2 == nkt or k0 + 4 == nkt:
                            # last two tile-pairs of this query block sit on
                            # the causal diagonal: zero out k > q entries
                            j0 = k0 - (nkt - npair)
                            nc.vector.tensor_mul(
                                pt, pt, mask_sb[:, j0 * qb : (j0 + 2) * qb]
                            )
                        for h in range(2):
                            ktile = k0 + h
                            nc.tensor.matmul(
                                nump,
                                lhsT=v_sb[:, ktile, :],
                                rhs=pt[:, h * qb : (h + 1) * qb],
                                start=(ktile == 0),
                                stop=(ktile == nkt - 1),
                            )
                    # row sums: ones-vector matmuls, stationary reused
                    for ktile in range(nkt):
                        pt = pts[ktile // 2]
                        h = ktile % 2
                        nc.tensor.matmul(
                            dp,
                            lhsT=ones_sb,
                            rhs=pt[:, h * qb : (h + 1) * qb],
                            start=(ktile == 0),
                            stop=(ktile == nkt - 1),
                        )
                    numo = out_pool.tile([128, qb], fp32, name="numo", tag="numo")
                    nc.vector.tensor_copy(numo, nump)
                    nc.sync.dma_start(out=num_out[:, qs], in_=numo)
                    nc.vector.tensor_copy(den_sb[:, qs], dp)
                nc.sync.dma_start(out=den_out[:, :], in_=den_sb)
    nc.compile()
    return nc


def _prep_inputs(x, Wq1, bq1, Wq2, bq2, Wk1, bk1, Wk2, bk2, Wv, bv):
    """Host-side data prep: fp16 transposed activations + weights. When all
    biases are zero (the standard case) skip the bias-fold augmentation row
    and its extra contraction chunk."""
    biases = [np.asarray(b, dtype=np.float32) for b in (bq1, bq2, bk1, bk2, bv)]
    need_aug = any(np.any(b) for b in biases)
    ea = EA if need_aug else E

    x = np.asarray(x, dtype=np.float32)
    xT = np.zeros((B, ea, S), dtype=np.float16)
    xT[:, :E, :] = x.transpose(0, 2, 1).astype(np.float16)
    if need_aug:
        xT[:, E, :] = 1.0  # ones row: folds the bias into the matmul

    def aug(W, b):
        Wa = np.zeros((ea, D), dtype=np.float16)
        Wa[:E] = np.asarray(W, dtype=np.float32).astype(np.float16)
        if need_aug:
            Wa[E] = np.asarray(b, dtype=np.float32).astype(np.float16)
        return Wa

    wq_br = [aug(Wq1, bq1), aug(Wq2, bq2)]
    wk_br = [aug(Wk1, bk1), aug(Wk2, bk2)]
    wv_a = aug(Wv, bv)

    # 0/1 masks for the diagonal tile-pairs, [128, 4*512] fp16:
    # variant j (kt = qb*4 + j): valid iff q_local >= j*128 + k_local
    ki = np.arange(KT)[:, None]
    qi = np.arange(QB)[None, :]
    dm = np.zeros((128, (QB // KT) * QB), dtype=np.float16)
    for j in range(QB // KT):
        dm[:, j * QB : (j + 1) * QB] = (qi >= j * KT + ki).astype(np.float16)
    return xT, wq_br, wk_br, wv_a, dm, ea


def kernel(x, Wq1, bq1, Wq2, bq2, Wk1, bk1, Wk2, bk2, Wv, bv, lam, mask):
    from concourse.bass_utils import run_bass_kernel_spmd

    xT, wq_br, wk_br, wv_a, dm, ea = _prep_inputs(
        x, Wq1, bq1, Wq2, bq2, Wk1, bk1, Wk2, bk2, Wv, bv
    )

    key = (S, ea, QB, KT)
    if key not in _PROG_CACHE:
        _PROG_CACHE[key] = _build_program(*key)
    nc = _PROG_CACHE[key]

    in_maps = []
    for c in range(8):
        b, br = c // 2, c % 2
        in_maps.append(
            {
                "xT": np.ascontiguousarray(xT[b]),
                "wq": wq_br[br],
                "wk": wk_br[br],
                "wv": wv_a,
                "dmask": dm,
            }
        )
    run = run_bass_kernel_spmd(nc, in_maps, core_ids=list(range(8)))
    global LAST_RUN
    LAST_RUN = run
    res = run.results

    lam = np.float32(np.asarray(lam))
    out = np.empty((B, S, D), dtype=np.float32)
    for b in range(B):
        n1, d1 = res[2 * b]["num"], res[2 * b]["den"]
        n2, d2 = res[2 * b + 1]["num"], res[2 * b + 1]["den"]
        out[b] = (n1 / d1 - lam * (n2 / d2)).T
    return out


# revision 14
# speedup vs baseline: 1.2301x; 1.0529x over previous
"""Differential self-attention head on 8 Trainium2 NeuronCores.

Sharding: 8 cores = 4 batches x 2 softmax branches. Core c handles batch
c//2 and branch c%2 (branch 0 -> (Wq1, Wk1), branch 1 -> (Wq2, Wk2)).
Every core runs the identical SPMD program over its own data:

  - projections q,k,v with bias folded in via an augmented contraction
    (E=1024 data rows + 1 ones-row + pad to 1152 = 9 chunks of 128)
  - causal scores computed transposed [k, q] so exp(S) is directly the
    moving operand of the v^T @ p matmul (no on-chip transpose of p)
  - exp on ScalarE straight from PSUM with scale=1/sqrt(D)
  - diagonal-tile causal masking via a multiply with host-built 0/1 tiles
  - row sums via ones-vector matmuls accumulated in PSUM
  - outputs the unnormalized numerator num = v^T @ p [D, S] and the
    denominator d [1, S]; the host divides and combines the two branches
    (o = num1/d1 - lam*num2/d2) and transposes back to [S, D].

All matmul operands are fp16 (measured end-to-end rel err ~7e-4);
accumulation is fp32 in PSUM.
"""

import sys

import numpy as np

for _p in ("/opt/trn_rl_repo",):
    if _p not in sys.path:
        sys.path.insert(0, _p)

B, S, E, D = 4, 4096, 1024, 128
EA = 1152  # augmented contraction: E + ones row, padded to 9*128
QB = 512  # query block (matmul moving free dim)
KT = 128  # key tile (partition dim of transposed scores)

_PROG_CACHE = {}
LAST_RUN = None  # BassKernelResults of the most recent kernel() call


def _build_program(s, ea, qb, kt):
    import concourse.bass as bass  # noqa: F401
    import concourse.mybir as mybir
    from concourse import bacc
    from concourse.tile import TileContext
    from concourse.masks import make_identity

    fp16 = mybir.dt.float16
    fp32 = mybir.dt.float32
    n_ec = ea // 128  # contraction chunks
    n_sb = s // qb  # 512-wide column blocks of the full sequence
    n_qb = s // qb  # query blocks
    n_st = s // kt  # 128-row key/seq tiles
    npair = qb // kt  # diag mask variants (kt tiles per query block)

    nc = bacc.Bacc("TRN2", target_bir_lowering=False, debug=False)
    xT = nc.dram_tensor("xT", [ea, s], fp16, kind="ExternalInput")
    wq = nc.dram_tensor("wq", [ea, D], fp16, kind="ExternalInput")
    wk = nc.dram_tensor("wk", [ea, D], fp16, kind="ExternalInput")
    wv = nc.dram_tensor("wv", [ea, D], fp16, kind="ExternalInput")
    dmask = nc.dram_tensor("dmask", [128, npair * qb], fp16, kind="ExternalInput")
    num_out = nc.dram_tensor("num", [D, s], fp32, kind="ExternalOutput")
    den_out = nc.dram_tensor("den", [1, s], fp32, kind="ExternalOutput")

    inv = 1.0 / np.sqrt(np.float32(D))

    with TileContext(nc) as tc:
        with (
            tc.tile_pool(name="const", bufs=1) as const_pool,
            tc.tile_pool(name="acts", bufs=1) as acts_pool,
            tc.tile_pool(name="ptiles", bufs=18) as p_pool,
            tc.tile_pool(name="outs", bufs=3) as out_pool,
        ):
            # ---- constants ----
            w_sb = const_pool.tile([128, n_ec, 3 * D], fp16, name="w_sb")
            nc.sync.dma_start(
                out=w_sb[:, :, 0:D], in_=wq.rearrange("(c p) d -> p c d", p=128)
            )
            nc.sync.dma_start(
                out=w_sb[:, :, D : 2 * D], in_=wk.rearrange("(c p) d -> p c d", p=128)
            )
            nc.sync.dma_start(
                out=w_sb[:, :, 2 * D : 3 * D],
                in_=wv.rearrange("(c p) d -> p c d", p=128),
            )
            ones_sb = const_pool.tile([128, 1], fp16, name="ones_sb")
            nc.vector.memset(ones_sb, 1.0)
            ident = const_pool.tile([128, 128], fp16, name="ident")
            make_identity(nc, ident)

            # ---- x^T staging: column-blocked, one DMA per (block, chunk) so
            # every HWDGE queue works on block 0 first and the projection
            # pipeline starts after ~1 MB of traffic, not ~8 MB ----
            xt_sb = acts_pool.tile([128, n_ec, s], fp16, name="xt_sb")
            mask_sb = const_pool.tile([128, npair * qb], fp16, name="mask_sb")
            for sb in range(n_sb):
                for c in range(n_ec):
                    nc.sync.dma_start(
                        out=xt_sb[:, c, sb * qb : (sb + 1) * qb],
                        in_=xT[c * 128 : (c + 1) * 128, sb * qb : (sb + 1) * qb],
                    )
                if sb == 0:
                    # masks aren't needed until the first diagonal tile;
                    # keep them out of the critical first-block window
                    nc.sync.dma_start(out=mask_sb, in_=dmask[:, :])

            # ---- projections (qT, kT, vT in [D, s] layout), sb-outer so
            # each column block completes as soon as its DMA lands ----
            qT = acts_pool.tile([128, s], fp16, name="qT")
            kTt = acts_pool.tile([128, s], fp16, name="kTt")
            v_sb = acts_pool.tile([128, n_st, D], fp16, name="v_sb")
            with (
                tc.tile_pool(name="proj_ps", bufs=2, space="PSUM") as proj_ps,
                tc.tile_pool(name="tr_ps", bufs=2, space="PSUM") as tr_ps,
            ):
                vT = acts_pool.tile([128, s], fp16, name="vT")
                for sb in range(n_sb):
                    for mi, dst in ((0, qT), (1, kTt), (2, vT)):
                        ps = proj_ps.tile([128, qb], fp32, name="ps", tag="ps")
                        for c in range(n_ec):
                            nc.tensor.matmul(
                                ps,
                                lhsT=w_sb[:, c, mi * D : (mi + 1) * D],
                                rhs=xt_sb[:, c, sb * qb : (sb + 1) * qb],
                                start=(c == 0),
                                stop=(c == n_ec - 1),
                            )
                        nc.vector.tensor_copy(dst[:, sb * qb : (sb + 1) * qb], ps)
                    # v natural layout [s, D] via PE transposes of vT
                    for j in range(qb // 128):
                        st = sb * (qb // 128) + j
                        tp = tr_ps.tile([128, 128], fp16, name="tp", tag="tp")
                        nc.tensor.transpose(
                            tp, vT[:, st * 128 : (st + 1) * 128], ident
                        )
                        nc.vector.tensor_copy(v_sb[:, st, :], tp)

            # ---- attention ----
            den_sb = out_pool.tile([1, s], fp32, name="den_sb", bufs=1)
            with (
                tc.tile_pool(name="s_ps", bufs=3, space="PSUM") as s_ps,
                tc.tile_pool(name="num_ps", bufs=1, space="PSUM") as num_ps,
                tc.tile_pool(name="d_ps", bufs=1, space="PSUM") as d_ps,
            ):
                for qbi in range(n_qb):
                    nkt = (qbi + 1) * (qb // kt)  # causal: key tiles needed
                    qs = slice(qbi * qb, (qbi + 1) * qb)
                    nump = num_ps.tile([128, qb], fp32, name="nump", tag="nump")
                    dp = d_ps.tile([1, qb], fp32, name="dp", tag="dp")
                    pts = []
                    for ktp in range(nkt // 2):
                        k0 = 2 * ktp
                        sp = s_ps.tile([128, 2 * qb], fp32, name="sp", tag="sp")
                        pt = p_pool.tile([128, 2 * qb], fp16, name="pt", tag="pt")
                        pts.append(pt)
                        for h in range(2):
                            ktile = k0 + h
                            nc.tensor.matmul(
                                sp[:, h * qb : (h + 1) * qb],
                                lhsT=kTt[:, ktile * kt : (ktile + 1) * kt],
                                rhs=qT[:, qs],
                                start=True,
                                stop=True,
                            )
                        nc.scalar.activation(
                            pt, sp, mybir.ActivationFunctionType.Exp, scale=float(inv)
                        )
                        if k0 + 2 == nkt or k0 + 4 == nkt:
                            # last two tile-pairs of this query block sit on
                            # the causal diagonal: zero out k > q entries
                            j0 = k0 - (nkt - npair)
                            nc.vector.tensor_mul(
                                pt, pt, mask_sb[:, j0 * qb : (j0 + 2) * qb]
                            )
                        for h in range(2):
                            ktile = k0 + h
                            nc.tensor.matmul(
                                nump,
                                lhsT=v_sb[:, ktile, :],
                                rhs=pt[:, h * qb : (h + 1) * qb],
                                start=(ktile == 0),
                                stop=(ktile == nkt - 1),
                            )
                    # row sums: ones-vector matmuls, stationary reused
                    for ktile in range(nkt):
                        pt = pts[ktile // 2]
                        h = ktile % 2
                        nc.tensor.matmul(
                            dp,
                            lhsT=ones_sb,
                            rhs=pt[:, h * qb : (h + 1) * qb],
                            start=(ktile == 0),
                            stop=(ktile == nkt - 1),
                        )
                    numo = out_pool.tile([128, qb], fp32, name="numo", tag="numo")
                    nc.vector.tensor_copy(numo, nump)
                    nc.sync.dma_start(out=num_out[:, qs], in_=numo)
                    nc.vector.tensor_copy(den_sb[:, qs], dp)
                nc.sync.dma_start(out=den_out[:, :], in_=den_sb)
    nc.compile()
    return nc


def _prep_inputs(x, Wq1, bq1, Wq2, bq2, Wk1, bk1, Wk2, bk2, Wv, bv):
    """Host-side data prep: fp16 transposed activations + weights. When all
    biases are zero (the standard case) skip the bias-fold augmentation row
    and its extra contraction chunk."""
    biases = [np.asarray(b, dtype=np.float32) for b in (bq1, bq2, bk1, bk2, bv)]
    need_aug = any(np.any(b) for b in biases)
    ea = EA if need_aug else E

    x = np.asarray(x, dtype=np.float32)
    xT = np.zeros((B, ea, S), dtype=np.float16)
    xT[:, :E, :] = x.transpose(0, 2, 1).astype(np.float16)
    if need_aug:
        xT[:, E, :] = 1.0  # ones row: folds the bias into the matmul

    def aug(W, b):
        Wa = np.zeros((ea, D), dtype=np.float16)
        Wa[:E] = np.asarray(W, dtype=np.float32).astype(np.float16)
        if need_aug:
            Wa[E] = np.asarray(b, dtype=np.float32).astype(np.float16)
        return Wa

    wq_br = [aug(Wq1, bq1), aug(Wq2, bq2)]
    wk_br = [aug(Wk1, bk1), aug(Wk2, bk2)]
    wv_a = aug(Wv, bv)

    # 0/1 masks for the diagonal tile-pairs, [128, 4*512] fp16:
    # variant j (kt = qb*4 + j): valid iff q_local >= j*128 + k_local
    ki = np.arange(KT)[:, None]
    qi = np.arange(QB)[None, :]
    dm = np.zeros((128, (QB // KT) * QB), dtype=np.float16)
    for j in range(QB // KT):
        dm[:, j * QB : (j + 1) * QB] = (qi >= j * KT + ki).astype(np.float16)
    return xT, wq_br, wk_br, wv_a, dm, ea


def kernel(x, Wq1, bq1, Wq2, bq2, Wk1, bk1, Wk2, bk2, Wv, bv, lam, mask):
    from concourse.bass_utils import run_bass_kernel_spmd

    xT, wq_br, wk_br, wv_a, dm, ea = _prep_inputs(
        x, Wq1, bq1, Wq2, bq2, Wk1, bk1, Wk2, bk2, Wv, bv
    )

    key = (S, ea, QB, KT)
    if key not in _PROG_CACHE:
        _PROG_CACHE[key] = _build_program(*key)
    nc = _PROG_CACHE[key]

    in_maps = []
    for c in range(8):
        b, br = c // 2, c % 2
        in_maps.append(
            {
                "xT": np.ascontiguousarray(xT[b]),
                "wq": wq_br[br],
                "wk": wk_br[br],
                "wv": wv_a,
                "dmask": dm,
            }
        )
    run = run_bass_kernel_spmd(nc, in_maps, core_ids=list(range(8)))
    global LAST_RUN
    LAST_RUN = run
    res = run.results

    lam = np.float32(np.asarray(lam))
    out = np.empty((B, S, D), dtype=np.float32)
    for b in range(B):
        n1, d1 = res[2 * b]["num"], res[2 * b]["den"]
        n2, d2 = res[2 * b + 1]["num"], res[2 * b + 1]["den"]
        out[b] = (n1 / d1 - lam * (n2 / d2)).T
    return out
